# revision 67
# baseline (speedup 1.0000x reference)
"""Multi-head causal attention (B=4, T=2048, C=1024, H=16) on 8 TRN2 NeuronCores.

Sharding: core c handles batch b=c//2 and head-group g=c%2 (8 heads = 4 pairs).
Per core: QKV projections for its 512 feature columns, causal attention for its
8 heads, out-projection accumulated over the 4 head pairs in PSUM. Host sums
the two head-group partials per batch and adds b_o.

All operands are bf16 (1 cyc/row matmuls + FWL weight loads + half the DMA
bytes); accumulation stays fp32 in PSUM.
"""
import sys
import numpy as np
from contextlib import ExitStack

sys.path.insert(0, "/opt/trn_rl_repo")

import concourse.bass as bass
import concourse.tile as tile
from concourse import bacc, mybir
from concourse.bass_utils import run_bass_kernel_spmd

f32 = mybir.dt.float32
bf16 = mybir.dt.bfloat16
fp8 = mybir.dt.float8e4
DR = mybir.MatmulPerfMode.DoubleRow
EXP = mybir.ActivationFunctionType.Exp

C = 1024          # model dim
HG = 512          # per-core head-group feature width (8 heads x 64)
D = 64            # head dim
DV = 66           # V' row: 64 V cols + ones col + pad (even => 4B-aligned heads)
NPAIR = 4         # head pairs per core
NCC = C // 128    # contraction chunks (8)
WS = 32.0         # host-side W_q/W_k fp8 scale (absorbed into the exp scale)
SCALE = 0.125     # 1/sqrt(D)
SCALE_S = SCALE / (WS * WS)   # exp scale for fp8-scaled Q/K scores


def build_kernel(T):
    """Emit the per-core Bass program. T = sequence length (multiple of 512)."""
    NQT = T // 512    # q tiles of 512
    NKT = T // 128    # k tiles of 128

    nc = bacc.Bacc("TRN2", target_bir_lowering=False, debug=False, num_devices=8)

    xT = nc.dram_tensor("xT", [C, T], bf16, kind="ExternalInput").ap()
    xT8 = nc.dram_tensor("xT8", [C, T], fp8, kind="ExternalInput").ap()
    wq = nc.dram_tensor("wq", [C, HG], fp8, kind="ExternalInput").ap()
    wk = nc.dram_tensor("wk", [C, HG], fp8, kind="ExternalInput").ap()
    wv = nc.dram_tensor("wv", [C, HG], bf16, kind="ExternalInput").ap()
    wo = nc.dram_tensor("wo", [HG, C], bf16, kind="ExternalInput").ap()
    out = nc.dram_tensor("out", [T, C], bf16, kind="ExternalOutput").ap()

    with tile.TileContext(nc) as tc, ExitStack() as ctx:
        # ---- SBUF pools (bytes/partition noted) ----
        p_xt = ctx.enter_context(tc.tile_pool(name="xt", bufs=NCC))          # 8x4K=32K
        p_w = ctx.enter_context(tc.tile_pool(name="w", bufs=2))              # qk pair weights 2x2x2K
        p_wv = ctx.enter_context(tc.tile_pool(name="wv", bufs=1))            # 8K
        p_wo = ctx.enter_context(tc.tile_pool(name="wo", bufs=NPAIR))        # 4x2K
        p_qk = ctx.enter_context(tc.tile_pool(name="qk", bufs=4))            # 4x4K
        p_v = ctx.enter_context(tc.tile_pool(name="v", bufs=4))              # 4x4.2K
        p_phat = ctx.enter_context(tc.tile_pool(name="phat", bufs=5))        # 5x2K
        p_ctxT = ctx.enter_context(tc.tile_pool(name="ctxT", bufs=4 * NPAIR))  # 16x1K
        p_cxs = ctx.enter_context(tc.tile_pool(name="cxs", bufs=4))          # 4x2K
        p_small = ctx.enter_context(tc.tile_pool(name="small", bufs=2))      # recip/bcast x2
        p_ostg = ctx.enter_context(tc.tile_pool(name="ostg", bufs=6))        # 6x2K
        p_ones = ctx.enter_context(tc.tile_pool(name="ones", bufs=1))
        # ---- PSUM pools: 4 + 2 + 2 = 8 banks ----
        ps_s = ctx.enter_context(tc.tile_pool(name="ps_s", bufs=2, space="PSUM"))    # [128,1024] x2
        ps_ctx = ctx.enter_context(tc.tile_pool(name="ps_ctx", bufs=1, space="PSUM"))
        ps_mm = ctx.enter_context(tc.tile_pool(name="ps_mm", bufs=2, space="PSUM"))

        # ---- constants + bulk loads ----
        ones_f = p_ones.tile([128, 1], bf16)
        nc.vector.memset(ones_f, 1.0)

        # A dummy exp preloads the ACT exp table set (~2.7us) before the
        # first real softmax. (PE warm-up dummies are emitted after the
        # first Q/K projections below -- those run during the cold window
        # while input DMAs are still in flight anyway.)
        warm = p_ostg.tile([128, 512], bf16, tag="ostg")
        nc.vector.memset(warm, 0.0)
        nc.scalar.activation(warm[:, 0:16], warm[:, 0:16], EXP, scale=1.0)

        # Input loads on the sync ring, ordered so each piece lands right
        # before its first consumer: Q/K projection needs xt8 (first q-tile
        # columns first), V needs wv + xt. Weights ride the scalar ring.
        def seg(lo, hi):
            return slice(lo, hi) if hi > lo else None
        xt8 = p_xt.tile([128, NCC, T], fp8, tag="xt8", bufs=1)
        xt = [p_xt.tile([128, T], bf16, tag="xt", name=f"xt{cc}")
              for cc in range(NCC)]
        wv_sb = p_wv.tile([128, NCC, HG], bf16)

        def load_x_cols(lo, hi, f8=True, fbf=True):
            if f8:
                nc.sync.dma_start(
                    xt8[:, :, lo:hi],
                    xT8[:, lo:hi].rearrange("(cc p) t -> p cc t", p=128))
            if fbf:
                for cc in range(NCC):
                    nc.sync.dma_start(xt[cc][:, lo:hi],
                                      xT[cc * 128 : (cc + 1) * 128, lo:hi])

        def load_wqk(p, eng=None):
            """[128, 8, 128] fp8 tile: cc-chunks of W{q,k}[:, p*128:(p+1)*128].

            Pair 0 loads on the sync ring AHEAD of the bulk x loads (the
            scalar ring is bandwidth-starved during startup, which would
            delay the first projections by ~10us)."""
            eng = eng or nc.scalar
            tq = p_w.tile([128, NCC, 128], fp8, tag="wq")
            tk = p_w.tile([128, NCC, 128], fp8, tag="wk")
            eng.dma_start(
                tq, wq[:, p * 128 : (p + 1) * 128].rearrange("(cc p) f -> p cc f", p=128))
            eng.dma_start(
                tk, wk[:, p * 128 : (p + 1) * 128].rearrange("(cc p) f -> p cc f", p=128))
            return tq, tk

        def load_wo(p):
            t_ = p_wo.tile([128, C], bf16, tag="wo", name=f"wo{p}")
            nc.scalar.dma_start(t_, wo[p * 128 : (p + 1) * 128, :])
            return t_

        # ---- filler unit generators (PE work to hide under ACT-bound attention) ----
        v_groups = [None] * (NKT // 4)   # [128, 4, 8, DV] tiles, 4 k-tiles each

        def v_tile(j):
            g = v_groups[j // 4]
            assert g is not None, f"V group {j // 4} not emitted yet"
            return g[:, j % 4]

        v_sb = [None] * NKT

        def v_unit(j):
            st = {}
            def emit_a():
                st["ps"] = ps_mm.tile([128, HG], f32, tag="mm", name="vps")
                for cc in range(NCC // 2):
                    nc.tensor.matmul(
                        st["ps"], xt[cc][:, j * 128 : (j + 1) * 128],
                        wv_sb[:, cc, :], start=(cc == 0), stop=False)
            def emit_b():
                ps = st["ps"]
                for cc in range(NCC // 2, NCC):
                    nc.tensor.matmul(
                        ps, xt[cc][:, j * 128 : (j + 1) * 128],
                        wv_sb[:, cc, :], start=False, stop=(cc == NCC - 1))
                # V' layout [128, 8 heads, DV]: 64 V columns + a ones column so a
                # single M=65 ctx matmul also produces the softmax denominator.
                if j % 4 == 0:
                    v_groups[j // 4] = p_v.tile(
                        [128, 4, 8, DV], bf16, tag="v", name=f"vg{j // 4}")
                t_ = v_tile(j)
                nc.vector.tensor_copy(
                    t_[:, :, 0:D], ps.rearrange("p (h d) -> p h d", h=8))
                nc.vector.tensor_copy(
                    t_[:, :, D : DV], ones_f.to_broadcast([128, 8, DV - D]))
                v_sb[j] = t_
            return [emit_a, emit_b]

        qkT = {}   # (('q'|'k'), pair) -> [128, T] tile

        def qk_unit(p, which, wtile, tt):
            st = {}
            def emit_a():
                key = (which, p)
                if key not in qkT:
                    qkT[key] = p_qk.tile([128, T], bf16, tag="qk", name=f"qk_{which}{p}")
                st["ps"] = ps_mm.tile([128, 512], f32, tag="mm", name="qkps")
                # fp8 DoubleRow: 2 contraction chunks per matmul (2 weights/cell)
                for cc in (0, 2):
                    nc.tensor.matmul(
                        st["ps"], wtile[:, cc : cc + 2, :],
                        xt8[:, cc : cc + 2, tt * 512 : (tt + 1) * 512],
                        start=(cc == 0), stop=False, perf_mode=DR)
            def emit_b():
                ps = st["ps"]
                for cc in (4, 6):
                    nc.tensor.matmul(
                        ps, wtile[:, cc : cc + 2, :],
                        xt8[:, cc : cc + 2, tt * 512 : (tt + 1) * 512],
                        start=False, stop=(cc == 6), perf_mode=DR)
                nc.vector.tensor_copy(qkT[(which, p)][:, tt * 512 : (tt + 1) * 512], ps)
            return [emit_a, emit_b]

        ctxT_store = {}  # (p, t) -> [128, 512] bf16 tile (normalized ctx^T)
        wo_tiles = {}

        def outproj_unit(t, qq, tail=False):
            """out[t*512+qq*128 : +128, :] = sum_p ctxT[p,t][:,qq]^T @ wo_p.

            Evictions go to the DVE; ONLY the post-attention tail units may
            use the scalar engine (mid-pair a scalar.copy sits in the ACT
            FIFO between exps and stalls the softmax chain while it waits
            for its own matmuls)."""
            st = {}
            def half(h):
                ps = ps_mm.tile([128, 512], f32, tag="mm")
                for p in range(NPAIR):
                    nc.tensor.matmul(
                        ps, ctxT_store[(p, t)][:, qq * 128 : (qq + 1) * 128],
                        wo_tiles[p][:, h * 512 : (h + 1) * 512],
                        start=(p == 0), stop=(p == NPAIR - 1))
                if tail and h == 1:
                    nc.scalar.copy(st["stg"][:, 512:1024], ps)
                else:
                    nc.vector.tensor_copy(st["stg"][:, h * 512 : (h + 1) * 512], ps)
            def emit_a():
                st["stg"] = p_ostg.tile([128, 1024], bf16, tag="ostg", name="ostg")
                half(0)
            def emit_b():
                half(1)
                nc.sync.dma_start(
                    out[t * 512 + qq * 128 : t * 512 + (qq + 1) * 128, :], st["stg"])
            return [emit_a, emit_b]

        pending_norm = []

        def make_norm(p, t, cxs):
            ct = p_ctxT.tile([128, 512], bf16, tag="ctxT", name=f"ct_{p}_{t}")
            ctxT_store[(p, t)] = ct
            def rep64(row):
                # [1,512] SBUF row -> [1, 64, 512] AP repeating the row 64x
                # (0-step on a free dim; partition dim must keep step!=0)
                return bass.AP(tensor=row.tensor, offset=row.offset,
                               ap=[list(row.ap[0]), [0, 64], list(row.ap[1])])
            state = {}
            # NOTE: all DMAs here ride the SYNC ring. nc.scalar.dma_start
            # issues from the ACT sequencer FIFO, so a dependent DMA chain
            # there stalls every exp queued behind it (~3-4us per tile).
            def front():
                sc = p_small.tile([64, 16], bf16, tag="scat")
                rc = p_small.tile([128, 1024], bf16, tag="recip")
                bcab = p_small.tile([64, 1024], bf16, tag="bcast")
                # scatter denom row over 64 lanes for the reciprocal
                # (serial per lane), gather, partition-broadcast on GPSIMD.
                nc.sync.dma_start(sc, cxs[64:65, :])
                with nc.allow_low_precision(reason="bf16 softmax recip, tol 2e-2"):
                    nc.vector.reciprocal(sc, sc)
                nc.sync.dma_start(rc[0:1, :], sc)
                # broadcast ring choice: rep64 DMAs would block the sync ring
                # behind the out-proj stores during the last pair; the gpsimd
                # path blocks the causal masks instead -- so switch per pair.
                if p == NPAIR - 1:
                    nc.gpsimd.partition_broadcast(bcab, rc[0:1, :])
                else:
                    nc.sync.dma_start(bcab[:, 0:512], rep64(rc[0:1, 0:512]))
                    nc.sync.dma_start(bcab[:, 512:1024], rep64(rc[0:1, 512:1024]))
                state["bc"], state["bc2"] = bcab[:, 0:512], bcab[:, 512:1024]
            def back():
                tmpB = p_small.tile([64, 512], bf16, tag="tmpB")
                nc.vector.tensor_mul(ct[0:64, :], cxs[0:64, 0:512], state["bc"])
                nc.vector.tensor_mul(tmpB, cxs[0:64, 512:1024], state["bc2"])
                nc.sync.dma_start(ct[64:128, :], tmpB)
            return p, t, front, back

        # ---- attention for one pair, pulling filler units between exp groups ----
        def attention(p, qt, kt, filler, t_order=None, pace_even=False):
            t_order = list(t_order or range(NQT))
            # pop slots per tile: one per j >= start threshold, +1 at tile end
            def tile_slots(ti, first):
                return (4 * (t_order[ti] + 1) - (1 if (first and ti == 0) else 3)) + 1
            first_pair = not pending_norm
            pace = {"acc": 0.0, "ratio": 2.0, "left": 1}
            def repace(ti):
                left = sum(tile_slots(k, first_pair) for k in range(ti, len(t_order)))
                pace["left"] = max(1, left)
                if pace_even:
                    pace["ratio"] = len(filler) / max(1, left)
            def pop_some():
                if pace_even:
                    pace["acc"] += pace["ratio"]
                    n = int(pace["acc"])
                    pace["acc"] -= n
                else:
                    n = 2
                for _ in range(min(n, len(filler))):
                    filler.pop(0)()
            for ti, t in enumerate(t_order):
                repace(ti)
                nk = 4 * (t + 1)
                norms = list(pending_norm)
                pending_norm.clear()
                for _, _, fr, _ in norms:
                    fr()
                cx = ps_ctx.tile([128, 1024], f32, tag="ctx")
                ctxA = cx[:, 0:512]
                ctxB = cx[:, 512:1024]
                for j in range(nk):
                    # run the deferred normalize muls mid-tile: at j==2 the
                    # scatter->recip->gather->broadcast chain (started at
                    # tile front) hasn't landed, and muls waiting at the DVE
                    # FIFO head block the psum-freeing casts behind them
                    if j == max(2, nk // 2):
                        for pp, tt, _, bk in norms:
                            bk()
                            if pp == NPAIR - 1:
                                for qq in range(4):
                                    filler.extend(outproj_unit(tt, qq))
                                repace(ti)
                    # causal narrowing: columns q < off are fully masked for
                    # this k-tile -> skip them in S, exp and ctx.
                    off = max(0, j * 128 - t * 512)
                    W = 512 - off
                    qs = t * 512 + off
                    # S^T for both heads, row-tiled (contraction d=64 each)
                    sps = ps_s.tile([128, 1024], f32, tag="s")
                    nc.tensor.matmul(
                        sps[:, off : 512], kt[0:64, j * 128 : (j + 1) * 128],
                        qt[0:64, qs : (t + 1) * 512],
                        start=True, stop=True, tile_position=(0, 0))
                    nc.tensor.matmul(
                        sps[:, 512 + off : 1024], kt[64:128, j * 128 : (j + 1) * 128],
                        qt[64:128, qs : (t + 1) * 512],
                        start=True, stop=True, tile_position=(64, 0))
                    # exp(scale * S^T) for both heads in one ACT instruction
                    # ([128, 2, W] AP skips the masked prefix columns)
                    ph = p_phat.tile([128, 1024], bf16, tag="phat")
                    nc.scalar.activation(
                        ph.rearrange("p (h w) -> p h w", h=2)[:, :, off:512],
                        sps.rearrange("p (h w) -> p h w", h=2)[:, :, off:512],
                        EXP, scale=SCALE_S)
                    # causal zeroing on the 128-col diagonal slab (q in
                    # [off, off+128)): standard lower-triangular mask.
                    if j * 128 + 127 > t * 512:  # block crosses the diagonal
                        oe = min(off + 128, 512)
                        for h in range(2):
                            nc.gpsimd.affine_select(
                                out=ph[:, h * 512 + off : h * 512 + oe],
                                in_=ph[:, h * 512 + off : h * 512 + oe],
                                compare_op=mybir.AluOpType.is_ge,
                                fill=0.0, base=0,
                                pattern=[[1, oe - off]], channel_multiplier=-1)
                    # ctx'^T accumulation: one M=DV matmul per head gives
                    # rows 0:64 = ctx^T and row 64 = softmax denominator
                    # (V' ones column). Single accumulation group per bank.
                    st, sp = (j == 0), (j == nk - 1)
                    assert v_sb[j] is not None, f"V tile {j} not emitted yet"
                    nc.tensor.matmul(ctxA[0:DV, off:512], v_sb[j][:, 2 * p, :],
                                     ph[:, off : 512], start=st, stop=sp)
                    nc.tensor.matmul(ctxB[0:DV, off:512], v_sb[j][:, 2 * p + 1, :],
                                     ph[:, 512 + off : 1024], start=st, stop=sp)
                    if j >= (3 if norms else 1):
                        pop_some()
                # Evict unnormalized ctx' (rows 0:64 ctx, row 64 denom) to
                # SBUF on the SCALAR engine (idle at tile boundaries, and it
                # bypasses the DVE queue) so the psum bank frees quickly.
                # The multi-hop normalize is deferred into the NEXT q-tile
                # iteration (front half at its start, muls at its middle) so
                # its DMA latency never heads the DVE queue.
                cxs = p_cxs.tile([128, 1024], bf16, tag="cxs")
                nc.vector.tensor_copy(cxs[0:65, :], cx[0:65, :])
                pending_norm.append(make_norm(p, t, cxs))
                if p == NPAIR - 1 and ti == len(t_order) - 1:
                    # very last tile: start its recip chain NOW so the tail
                    # only pays the muls + out-proj, not the whole
                    # scatter->recip->gather->broadcast latency
                    pp_, tt_, fr_, bk_ = pending_norm.pop()
                    fr_()
                    pending_norm.append((pp_, tt_, lambda: None, bk_))
                pop_some()

        # ================= emission schedule =================
        # V tiles 0..3 + pair-0 Q/K tile 0 upfront; pair-0's later Q/K tiles,
        # the rest of V, later pairs' proj and the accumulated out-proj are
        # filler inside the attention loops (ordered so each is emitted
        # before its first use -- the v_sb/qkT asserts verify this).
        w0q, w0k = load_wqk(0, eng=nc.gpsimd)
        load_x_cols(0, min(512, T), fbf=False)            # Q/K tile 0
        nc.sync.dma_start(wv_sb, wv.rearrange("(cc p) f -> p cc f", p=128))
        load_x_cols(0, min(512, T), f8=False)             # V k-tiles 0..3
        if T > 512:
            load_x_cols(512, T, fbf=False)                # Q/K tiles 1+
            load_x_cols(512, T, f8=False)                 # V k-tiles 4+
        for piece in qk_unit(0, "q", w0q, 0) + qk_unit(0, "k", w0k, 0):
            piece()
        # HAM warm-up after the first projections: keeps the PE busy while
        # the V-path inputs (wv + xT) are still in flight.
        wps = ps_mm.tile([128, 512], f32, tag="mm")
        for i in range(10):
            nc.tensor.matmul(wps, warm[:, 0:128], warm,
                             start=(i == 0), stop=(i == 9))
        for j in range(4 * 1):
            for piece in v_unit(j):
                piece()

        for p in range(NPAIR):
            filler = []
            if p == 0:
                vq = [v_unit(j) for j in range(4, NKT)]
                for tt in range(1, NQT):
                    filler.extend(qk_unit(0, "q", w0q, tt))
                    filler.extend(qk_unit(0, "k", w0k, tt))
                    for u in vq[:2]:
                        filler.extend(u)
                    vq = vq[2:]
                for u in vq:
                    filler.extend(u)
            if p + 1 < NPAIR:
                wq_t, wk_t = load_wqk(p + 1)
                for tt in range(NQT):
                    filler.extend(qk_unit(p + 1, "q", wq_t, tt))
                    filler.extend(qk_unit(p + 1, "k", wk_t, tt))
            wo_tiles[p] = load_wo(p)
            # last pair ends on the smallest q-tile so the un-hideable
            # exp tail + final out-proj is as short as possible
            t_order = None
            if p == NPAIR - 1 and NQT >= 3:
                t_order = [NQT - 2, NQT - 1] + list(range(NQT - 3, -1, -1))
            attention(p, qkT[("q", p)], qkT[("k", p)], filler, t_order,
                      pace_even=(p > 0))
            for u in filler:  # drain any leftovers
                u()
            qkT.pop(("q", p)), qkT.pop(("k", p))
        # tail: last tile's normalize + its out-projection
        for pp, tt, fr, bk in pending_norm:
            fr(); bk()
            if pp == NPAIR - 1:
                for qq in range(4):
                    for piece in outproj_unit(tt, qq, tail=True):
                        piece()
        pending_norm.clear()

    nc.compile()
    return nc


_NC_CACHE = {}


def _get_nc(T):
    if T not in _NC_CACHE:
        _NC_CACHE[T] = build_kernel(T)
    return _NC_CACHE[T]


def _bf16(a):
    import ml_dtypes
    return np.ascontiguousarray(a).astype(ml_dtypes.bfloat16)


def _fp8(a):
    import ml_dtypes
    return np.ascontiguousarray(a).astype(ml_dtypes.float8_e4m3)


def make_in_maps(x, W_q, W_k, W_v, W_o):
    B, T, _ = x.shape
    in_maps = []
    for c in range(8):
        b, g = c // 2, c % 2
        cols = slice(g * HG, (g + 1) * HG)
        xTb = np.asarray(x[b]).T
        in_maps.append({
            "xT": _bf16(xTb),
            "xT8": _fp8(xTb),
            "wq": _fp8(np.asarray(W_q)[:, cols] * WS),
            "wk": _fp8(np.asarray(W_k)[:, cols] * WS),
            "wv": _bf16(np.asarray(W_v)[:, cols]),
            "wo": _bf16(np.asarray(W_o)[cols, :]),
        })
    return in_maps


def kernel(x, W_q, W_k, W_v, W_o, b_o):
    x = np.asarray(x, dtype=np.float32)
    B, T, C_ = x.shape
    nc = _get_nc(T)
    in_maps = make_in_maps(x, W_q, W_k, W_v, W_o)
    res = run_bass_kernel_spmd(nc, in_maps, core_ids=list(range(8)))
    out = np.empty((B, T, C_), dtype=np.float32)
    bo = np.asarray(b_o, dtype=np.float32)[None, :]
    for b in range(B):
        pa = np.asarray(res.results[2 * b]["out"]).astype(np.float32)
        pb = np.asarray(res.results[2 * b + 1]["out"]).astype(np.float32)
        out[b] = pa + pb + bo
    return out


# revision 69
# speedup vs baseline: 1.0108x; 1.0108x over previous
"""Multi-head causal attention (B=4, T=2048, C=1024, H=16) on 8 TRN2 NeuronCores.

Sharding: core c handles batch b=c//2 and head-group g=c%2 (8 heads = 4 pairs).
Per core: QKV projections for its 512 feature columns, causal attention for its
8 heads, out-projection accumulated over the 4 head pairs in PSUM. Host sums
the two head-group partials per batch and adds b_o.

All operands are bf16 (1 cyc/row matmuls + FWL weight loads + half the DMA
bytes); accumulation stays fp32 in PSUM.
"""
import sys
import numpy as np
from contextlib import ExitStack

sys.path.insert(0, "/opt/trn_rl_repo")

import concourse.bass as bass
import concourse.tile as tile
from concourse import bacc, mybir
from concourse.bass_utils import run_bass_kernel_spmd

f32 = mybir.dt.float32
bf16 = mybir.dt.bfloat16
fp8 = mybir.dt.float8e4
DR = mybir.MatmulPerfMode.DoubleRow
EXP = mybir.ActivationFunctionType.Exp

C = 1024          # model dim
HG = 512          # per-core head-group feature width (8 heads x 64)
D = 64            # head dim
DV = 66           # V' row: 64 V cols + ones col + pad (even => 4B-aligned heads)
NPAIR = 4         # head pairs per core
NCC = C // 128    # contraction chunks (8)
WS = 32.0         # host-side W_q/W_k fp8 scale (absorbed into the exp scale)
SCALE = 0.125     # 1/sqrt(D)
SCALE_S = SCALE / (WS * WS)   # exp scale for fp8-scaled Q/K scores


def build_kernel(T):
    """Emit the per-core Bass program. T = sequence length (multiple of 512)."""
    NQT = T // 512    # q tiles of 512
    NKT = T // 128    # k tiles of 128

    nc = bacc.Bacc("TRN2", target_bir_lowering=False, debug=False, num_devices=8)

    xT = nc.dram_tensor("xT", [C, T], bf16, kind="ExternalInput").ap()
    xT8 = nc.dram_tensor("xT8", [C, T], fp8, kind="ExternalInput").ap()
    wq = nc.dram_tensor("wq", [C, HG], fp8, kind="ExternalInput").ap()
    wk = nc.dram_tensor("wk", [C, HG], fp8, kind="ExternalInput").ap()
    wv = nc.dram_tensor("wv", [C, HG], bf16, kind="ExternalInput").ap()
    wo = nc.dram_tensor("wo", [HG, C], bf16, kind="ExternalInput").ap()
    out = nc.dram_tensor("out", [T, C], bf16, kind="ExternalOutput").ap()

    with tile.TileContext(nc) as tc, ExitStack() as ctx:
        # ---- SBUF pools (bytes/partition noted) ----
        p_xt = ctx.enter_context(tc.tile_pool(name="xt", bufs=NCC))          # 8x4K=32K
        p_w = ctx.enter_context(tc.tile_pool(name="w", bufs=2))              # qk pair weights 2x2x2K
        p_wv = ctx.enter_context(tc.tile_pool(name="wv", bufs=1))            # 8K
        p_wo = ctx.enter_context(tc.tile_pool(name="wo", bufs=NPAIR))        # 4x2K
        p_qk = ctx.enter_context(tc.tile_pool(name="qk", bufs=4))            # 4x4K
        p_v = ctx.enter_context(tc.tile_pool(name="v", bufs=4))              # 4x4.2K
        p_phat = ctx.enter_context(tc.tile_pool(name="phat", bufs=5))        # 5x2K
        p_ctxT = ctx.enter_context(tc.tile_pool(name="ctxT", bufs=4 * NPAIR))  # 16x1K
        p_cxs = ctx.enter_context(tc.tile_pool(name="cxs", bufs=4))          # 4x2K
        p_small = ctx.enter_context(tc.tile_pool(name="small", bufs=2))      # recip/bcast x2
        p_ostg = ctx.enter_context(tc.tile_pool(name="ostg", bufs=6))        # 6x2K
        p_ones = ctx.enter_context(tc.tile_pool(name="ones", bufs=1))
        # ---- PSUM pools: 4 + 2 + 2 = 8 banks ----
        ps_s = ctx.enter_context(tc.tile_pool(name="ps_s", bufs=2, space="PSUM"))    # [128,1024] x2
        ps_ctx = ctx.enter_context(tc.tile_pool(name="ps_ctx", bufs=1, space="PSUM"))
        ps_mm = ctx.enter_context(tc.tile_pool(name="ps_mm", bufs=2, space="PSUM"))

        # ---- constants + bulk loads ----
        ones_f = p_ones.tile([128, 1], bf16)
        nc.vector.memset(ones_f, 1.0)

        # A dummy exp preloads the ACT exp table set (~2.7us) before the
        # first real softmax. (PE warm-up dummies are emitted after the
        # first Q/K projections below -- those run during the cold window
        # while input DMAs are still in flight anyway.)
        warm = p_ostg.tile([128, 512], bf16, tag="ostg")
        nc.vector.memset(warm, 0.0)
        nc.scalar.activation(warm[:, 0:16], warm[:, 0:16], EXP, scale=1.0)

        # Input loads on the sync ring, ordered so each piece lands right
        # before its first consumer: Q/K projection needs xt8 (first q-tile
        # columns first), V needs wv + xt. Weights ride the scalar ring.
        def seg(lo, hi):
            return slice(lo, hi) if hi > lo else None
        xt8 = p_xt.tile([128, NCC, T], fp8, tag="xt8", bufs=1)
        xt = [p_xt.tile([128, T], bf16, tag="xt", name=f"xt{cc}")
              for cc in range(NCC)]
        wv_sb = p_wv.tile([128, NCC, HG], bf16)

        def load_x_cols(lo, hi, f8=True, fbf=True):
            if f8:
                nc.sync.dma_start(
                    xt8[:, :, lo:hi],
                    xT8[:, lo:hi].rearrange("(cc p) t -> p cc t", p=128))
            if fbf:
                for cc in range(NCC):
                    nc.sync.dma_start(xt[cc][:, lo:hi],
                                      xT[cc * 128 : (cc + 1) * 128, lo:hi])

        def load_wqk(p, eng=None):
            """[128, 8, 128] fp8 tile: cc-chunks of W{q,k}[:, p*128:(p+1)*128].

            Pair 0 loads on the sync ring AHEAD of the bulk x loads (the
            scalar ring is bandwidth-starved during startup, which would
            delay the first projections by ~10us)."""
            eng = eng or nc.scalar
            tq = p_w.tile([128, NCC, 128], fp8, tag="wq")
            tk = p_w.tile([128, NCC, 128], fp8, tag="wk")
            eng.dma_start(
                tq, wq[:, p * 128 : (p + 1) * 128].rearrange("(cc p) f -> p cc f", p=128))
            eng.dma_start(
                tk, wk[:, p * 128 : (p + 1) * 128].rearrange("(cc p) f -> p cc f", p=128))
            return tq, tk

        def load_wo(p):
            t_ = p_wo.tile([128, C], bf16, tag="wo", name=f"wo{p}")
            nc.scalar.dma_start(t_, wo[p * 128 : (p + 1) * 128, :])
            return t_

        # ---- filler unit generators (PE work to hide under ACT-bound attention) ----
        v_groups = [None] * (NKT // 4)   # [128, 4, 8, DV] tiles, 4 k-tiles each

        def v_tile(j):
            g = v_groups[j // 4]
            assert g is not None, f"V group {j // 4} not emitted yet"
            return g[:, j % 4]

        v_sb = [None] * NKT

        def v_unit(j):
            st = {}
            def emit_a():
                st["ps"] = ps_mm.tile([128, HG], f32, tag="mm", name="vps")
                for cc in range(NCC // 2):
                    nc.tensor.matmul(
                        st["ps"], xt[cc][:, j * 128 : (j + 1) * 128],
                        wv_sb[:, cc, :], start=(cc == 0), stop=False)
            def emit_b():
                ps = st["ps"]
                for cc in range(NCC // 2, NCC):
                    nc.tensor.matmul(
                        ps, xt[cc][:, j * 128 : (j + 1) * 128],
                        wv_sb[:, cc, :], start=False, stop=(cc == NCC - 1))
                # V' layout [128, 8 heads, DV]: 64 V columns + a ones column so a
                # single M=65 ctx matmul also produces the softmax denominator.
                if j % 4 == 0:
                    v_groups[j // 4] = p_v.tile(
                        [128, 4, 8, DV], bf16, tag="v", name=f"vg{j // 4}")
                t_ = v_tile(j)
                nc.vector.tensor_copy(
                    t_[:, :, 0:D], ps.rearrange("p (h d) -> p h d", h=8))
                nc.vector.tensor_copy(
                    t_[:, :, D : DV], ones_f.to_broadcast([128, 8, DV - D]))
                v_sb[j] = t_
            return [emit_a, emit_b]

        qkT = {}   # (('q'|'k'), pair) -> [128, T] tile

        def qk_unit(p, which, wtile, tt):
            st = {}
            def emit_a():
                key = (which, p)
                if key not in qkT:
                    qkT[key] = p_qk.tile([128, T], bf16, tag="qk", name=f"qk_{which}{p}")
                st["ps"] = ps_mm.tile([128, 512], f32, tag="mm", name="qkps")
                # fp8 DoubleRow: 2 contraction chunks per matmul (2 weights/cell)
                for cc in (0, 2):
                    nc.tensor.matmul(
                        st["ps"], wtile[:, cc : cc + 2, :],
                        xt8[:, cc : cc + 2, tt * 512 : (tt + 1) * 512],
                        start=(cc == 0), stop=False, perf_mode=DR)
            def emit_b():
                ps = st["ps"]
                for cc in (4, 6):
                    nc.tensor.matmul(
                        ps, wtile[:, cc : cc + 2, :],
                        xt8[:, cc : cc + 2, tt * 512 : (tt + 1) * 512],
                        start=False, stop=(cc == 6), perf_mode=DR)
                nc.vector.tensor_copy(qkT[(which, p)][:, tt * 512 : (tt + 1) * 512], ps)
            return [emit_a, emit_b]

        ctxT_store = {}  # (p, t) -> [128, 512] bf16 tile (normalized ctx^T)
        wo_tiles = {}

        def outproj_unit(t, qq, tail=False):
            """out[t*512+qq*128 : +128, :] = sum_p ctxT[p,t][:,qq]^T @ wo_p.

            Evictions go to the DVE; ONLY the post-attention tail units may
            use the scalar engine (mid-pair a scalar.copy sits in the ACT
            FIFO between exps and stalls the softmax chain while it waits
            for its own matmuls)."""
            st = {}
            def half(h):
                ps = ps_mm.tile([128, 512], f32, tag="mm")
                for p in range(NPAIR):
                    nc.tensor.matmul(
                        ps, ctxT_store[(p, t)][:, qq * 128 : (qq + 1) * 128],
                        wo_tiles[p][:, h * 512 : (h + 1) * 512],
                        start=(p == 0), stop=(p == NPAIR - 1))
                if tail and h == 1:
                    nc.scalar.copy(st["stg"][:, 512:1024], ps)
                else:
                    nc.vector.tensor_copy(st["stg"][:, h * 512 : (h + 1) * 512], ps)
            def emit_a():
                st["stg"] = p_ostg.tile([128, 1024], bf16, tag="ostg", name="ostg")
                half(0)
            def emit_b():
                half(1)
                nc.sync.dma_start(
                    out[t * 512 + qq * 128 : t * 512 + (qq + 1) * 128, :], st["stg"])
            return [emit_a, emit_b]

        pending_norm = []

        def make_norm(p, t, cxs):
            ct = p_ctxT.tile([128, 512], bf16, tag="ctxT", name=f"ct_{p}_{t}")
            ctxT_store[(p, t)] = ct
            def rep64(row):
                # [1,512] SBUF row -> [1, 64, 512] AP repeating the row 64x
                # (0-step on a free dim; partition dim must keep step!=0)
                return bass.AP(tensor=row.tensor, offset=row.offset,
                               ap=[list(row.ap[0]), [0, 64], list(row.ap[1])])
            state = {}
            # NOTE: all DMAs here ride the SYNC ring. nc.scalar.dma_start
            # issues from the ACT sequencer FIFO, so a dependent DMA chain
            # there stalls every exp queued behind it (~3-4us per tile).
            def front():
                sc = p_small.tile([64, 16], bf16, tag="scat")
                rc = p_small.tile([128, 1024], bf16, tag="recip")
                bcab = p_small.tile([64, 1024], bf16, tag="bcast")
                # scatter denom row over 64 lanes for the reciprocal
                # (serial per lane), gather, partition-broadcast on GPSIMD.
                nc.sync.dma_start(sc, cxs[64:65, :])
                with nc.allow_low_precision(reason="bf16 softmax recip, tol 2e-2"):
                    nc.vector.reciprocal(sc, sc)
                nc.sync.dma_start(rc[0:1, :], sc)
                # rep64 broadcast on the sync ring for every pair: with the
                # muls deferred to mid-tile the chain never heads a queue,
                # and the gpsimd path costs 2.1us right where pair 3 stalls
                nc.sync.dma_start(bcab[:, 0:512], rep64(rc[0:1, 0:512]))
                nc.sync.dma_start(bcab[:, 512:1024], rep64(rc[0:1, 512:1024]))
                state["bc"], state["bc2"] = bcab[:, 0:512], bcab[:, 512:1024]
            def back():
                tmpB = p_small.tile([64, 512], bf16, tag="tmpB")
                nc.vector.tensor_mul(ct[0:64, :], cxs[0:64, 0:512], state["bc"])
                nc.vector.tensor_mul(tmpB, cxs[0:64, 512:1024], state["bc2"])
                nc.sync.dma_start(ct[64:128, :], tmpB)
            return p, t, front, back

        # ---- attention for one pair, pulling filler units between exp groups ----
        def attention(p, qt, kt, filler, t_order=None, pace_even=False):
            t_order = list(t_order or range(NQT))
            # pop slots per tile: one per j >= start threshold, +1 at tile end
            def tile_slots(ti, first):
                return (4 * (t_order[ti] + 1) - (1 if (first and ti == 0) else 3)) + 1
            first_pair = not pending_norm
            pace = {"acc": 0.0, "ratio": 2.0, "left": 1}
            def repace(ti):
                left = sum(tile_slots(k, first_pair) for k in range(ti, len(t_order)))
                pace["left"] = max(1, left)
                if pace_even:
                    pace["ratio"] = len(filler) / max(1, left)
            def pop_some():
                if pace_even:
                    pace["acc"] += pace["ratio"]
                    n = int(pace["acc"])
                    pace["acc"] -= n
                else:
                    n = 2
                for _ in range(min(n, len(filler))):
                    filler.pop(0)()
            for ti, t in enumerate(t_order):
                repace(ti)
                nk = 4 * (t + 1)
                norms = list(pending_norm)
                pending_norm.clear()
                for _, _, fr, _ in norms:
                    fr()
                cx = ps_ctx.tile([128, 1024], f32, tag="ctx")
                ctxA = cx[:, 0:512]
                ctxB = cx[:, 512:1024]
                for j in range(nk):
                    # run the deferred normalize muls mid-tile: at j==2 the
                    # scatter->recip->gather->broadcast chain (started at
                    # tile front) hasn't landed, and muls waiting at the DVE
                    # FIFO head block the psum-freeing casts behind them
                    if j == max(2, nk // 2):
                        for pp, tt, _, bk in norms:
                            bk()
                            if pp == NPAIR - 1:
                                for qq in range(4):
                                    filler.extend(outproj_unit(tt, qq))
                                repace(ti)
                    # causal narrowing: columns q < off are fully masked for
                    # this k-tile -> skip them in S, exp and ctx.
                    off = max(0, j * 128 - t * 512)
                    W = 512 - off
                    qs = t * 512 + off
                    # S^T for both heads, row-tiled (contraction d=64 each)
                    sps = ps_s.tile([128, 1024], f32, tag="s")
                    nc.tensor.matmul(
                        sps[:, off : 512], kt[0:64, j * 128 : (j + 1) * 128],
                        qt[0:64, qs : (t + 1) * 512],
                        start=True, stop=True, tile_position=(0, 0))
                    nc.tensor.matmul(
                        sps[:, 512 + off : 1024], kt[64:128, j * 128 : (j + 1) * 128],
                        qt[64:128, qs : (t + 1) * 512],
                        start=True, stop=True, tile_position=(64, 0))
                    # exp(scale * S^T) for both heads in one ACT instruction
                    # ([128, 2, W] AP skips the masked prefix columns)
                    ph = p_phat.tile([128, 1024], bf16, tag="phat")
                    nc.scalar.activation(
                        ph.rearrange("p (h w) -> p h w", h=2)[:, :, off:512],
                        sps.rearrange("p (h w) -> p h w", h=2)[:, :, off:512],
                        EXP, scale=SCALE_S)
                    # causal zeroing on the 128-col diagonal slab (q in
                    # [off, off+128)): standard lower-triangular mask.
                    if j * 128 + 127 > t * 512:  # block crosses the diagonal
                        oe = min(off + 128, 512)
                        for h in range(2):
                            nc.gpsimd.affine_select(
                                out=ph[:, h * 512 + off : h * 512 + oe],
                                in_=ph[:, h * 512 + off : h * 512 + oe],
                                compare_op=mybir.AluOpType.is_ge,
                                fill=0.0, base=0,
                                pattern=[[1, oe - off]], channel_multiplier=-1)
                    # ctx'^T accumulation: one M=DV matmul per head gives
                    # rows 0:64 = ctx^T and row 64 = softmax denominator
                    # (V' ones column). Single accumulation group per bank.
                    st, sp = (j == 0), (j == nk - 1)
                    assert v_sb[j] is not None, f"V tile {j} not emitted yet"
                    nc.tensor.matmul(ctxA[0:DV, off:512], v_sb[j][:, 2 * p, :],
                                     ph[:, off : 512], start=st, stop=sp)
                    nc.tensor.matmul(ctxB[0:DV, off:512], v_sb[j][:, 2 * p + 1, :],
                                     ph[:, 512 + off : 1024], start=st, stop=sp)
                    if j >= (3 if norms else 1):
                        pop_some()
                # Evict unnormalized ctx' (rows 0:64 ctx, row 64 denom) to
                # SBUF on the SCALAR engine (idle at tile boundaries, and it
                # bypasses the DVE queue) so the psum bank frees quickly.
                # The multi-hop normalize is deferred into the NEXT q-tile
                # iteration (front half at its start, muls at its middle) so
                # its DMA latency never heads the DVE queue.
                cxs = p_cxs.tile([128, 1024], bf16, tag="cxs")
                nc.vector.tensor_copy(cxs[0:65, :], cx[0:65, :])
                pending_norm.append(make_norm(p, t, cxs))
                pop_some()

        # ================= emission schedule =================
        # V tiles 0..3 + pair-0 Q/K tile 0 upfront; pair-0's later Q/K tiles,
        # the rest of V, later pairs' proj and the accumulated out-proj are
        # filler inside the attention loops (ordered so each is emitted
        # before its first use -- the v_sb/qkT asserts verify this).
        w0q, w0k = load_wqk(0, eng=nc.gpsimd)
        load_x_cols(0, min(512, T), fbf=False)            # Q/K tile 0
        nc.sync.dma_start(wv_sb, wv.rearrange("(cc p) f -> p cc f", p=128))
        load_x_cols(0, min(512, T), f8=False)             # V k-tiles 0..3
        if T > 512:
            load_x_cols(512, T, fbf=False)                # Q/K tiles 1+
            load_x_cols(512, T, f8=False)                 # V k-tiles 4+
        for piece in qk_unit(0, "q", w0q, 0) + qk_unit(0, "k", w0k, 0):
            piece()
        # HAM warm-up after the first projections: keeps the PE busy while
        # the V-path inputs (wv + xT) are still in flight.
        wps = ps_mm.tile([128, 512], f32, tag="mm")
        for i in range(10):
            nc.tensor.matmul(wps, warm[:, 0:128], warm,
                             start=(i == 0), stop=(i == 9))
        for j in range(4 * 1):
            for piece in v_unit(j):
                piece()

        for p in range(NPAIR):
            filler = []
            if p == 0:
                vq = [v_unit(j) for j in range(4, NKT)]
                for tt in range(1, NQT):
                    filler.extend(qk_unit(0, "q", w0q, tt))
                    filler.extend(qk_unit(0, "k", w0k, tt))
                    for u in vq[:2]:
                        filler.extend(u)
                    vq = vq[2:]
                for u in vq:
                    filler.extend(u)
            if p + 1 < NPAIR:
                wq_t, wk_t = load_wqk(p + 1)
                for tt in range(NQT):
                    filler.extend(qk_unit(p + 1, "q", wq_t, tt))
                    filler.extend(qk_unit(p + 1, "k", wk_t, tt))
            wo_tiles[p] = load_wo(p)
            # last pair ends on the smallest q-tile so the un-hideable
            # exp tail + final out-proj is as short as possible
            t_order = None
            if p == NPAIR - 1 and NQT >= 3:
                t_order = [NQT - 2, NQT - 1] + list(range(NQT - 3, -1, -1))
            attention(p, qkT[("q", p)], qkT[("k", p)], filler, t_order,
                      pace_even=(p > 0))
            for u in filler:  # drain any leftovers
                u()
            qkT.pop(("q", p)), qkT.pop(("k", p))
        # tail: last tile's normalize + its out-projection
        for pp, tt, fr, bk in pending_norm:
            fr(); bk()
            if pp == NPAIR - 1:
                for qq in range(4):
                    for piece in outproj_unit(tt, qq, tail=True):
                        piece()
        pending_norm.clear()

    nc.compile()
    return nc


_NC_CACHE = {}


def _get_nc(T):
    if T not in _NC_CACHE:
        _NC_CACHE[T] = build_kernel(T)
    return _NC_CACHE[T]


def _bf16(a):
    import ml_dtypes
    return np.ascontiguousarray(a).astype(ml_dtypes.bfloat16)


def _fp8(a):
    import ml_dtypes
    return np.ascontiguousarray(a).astype(ml_dtypes.float8_e4m3)


def make_in_maps(x, W_q, W_k, W_v, W_o):
    B, T, _ = x.shape
    in_maps = []
    for c in range(8):
        b, g = c // 2, c % 2
        cols = slice(g * HG, (g + 1) * HG)
        xTb = np.asarray(x[b]).T
        in_maps.append({
            "xT": _bf16(xTb),
            "xT8": _fp8(xTb),
            "wq": _fp8(np.asarray(W_q)[:, cols] * WS),
            "wk": _fp8(np.asarray(W_k)[:, cols] * WS),
            "wv": _bf16(np.asarray(W_v)[:, cols]),
            "wo": _bf16(np.asarray(W_o)[cols, :]),
        })
    return in_maps


def kernel(x, W_q, W_k, W_v, W_o, b_o):
    x = np.asarray(x, dtype=np.float32)
    B, T, C_ = x.shape
    nc = _get_nc(T)
    in_maps = make_in_maps(x, W_q, W_k, W_v, W_o)
    res = run_bass_kernel_spmd(nc, in_maps, core_ids=list(range(8)))
    out = np.empty((B, T, C_), dtype=np.float32)
    bo = np.asarray(b_o, dtype=np.float32)[None, :]
    for b in range(B):
        pa = np.asarray(res.results[2 * b]["out"]).astype(np.float32)
        pb = np.asarray(res.results[2 * b + 1]["out"]).astype(np.float32)
        out[b] = pa + pb + bo
    return out
